# revision 21
# baseline (speedup 1.0000x reference)
"""Randomized Hadamard transform kernel for Trainium2 (8 NeuronCores, SPMD).

Math: out = FWHT(x * seed) / sqrt(4096); the reference butterfly equals the
Sylvester Hadamard matrix, and H_4096 = H_32 (x) H_128 (c = hi*128 + lo):

    out[r, j*128 + l] = (1/64) * sum_{hi,lo} H32[hi,j] H128[lo,l] x~[r, hi*128+lo]

Layout trick: matmul computes out[m, n] = sum_k lhsT[k, m] * rhs[k, n].
With the *data* as stationary lhsT and the Hadamard factor as moving rhs,
one MM both contracts the data's partition index and rotates a 128-wide
free window onto the output partitions. Two such passes apply both factors
and land in a store-friendly orientation — no explicit transposes.

This version keeps the baseline's matmul structure but moves the whole
datapath to fp16 (PE: 1 cycle/row vs fp32's 4; DMA bytes halved) and hoists
the seed multiply plus both layout shuffles to the host, so every device DMA
is a fully contiguous [128 x 4096] block (8 KB per partition line):

  host   xp[(t,rl,hi), (rh,lo)] = x[128t+4rh+rl, 128hi+lo] * seed[128hi+lo]
  tile   load xt <- xp  (1 DMA, contiguous)
  pass1  psum[lo, (rl,j)]  = sum_(rl,hi) xt * (I4 (x) H32)   per rh
  pass2  psum[(rl,j), l]   = sum_lo    w  * (H128/64)        per rh
  store  yp[(t,rl,j), (rh,l)] -> out  (1 DMA, contiguous)
  host   out[128t+4rh+rl, 128j+l] = yp

Pass2 of tile t is issued after pass1 of tile t+1 (one-tile software skew)
so the in-order PE queue never waits on the PSUM-drain copies, which are
rotated across the DVE / Activation / Pool engines.
"""

import numpy as np

import concourse.mybir as mybir
from concourse import bacc
import concourse.tile as tile
from concourse.bass_utils import run_bass_kernel_spmd

N_CORES = 8
R_FULL = 8192
C = 4096
R_CORE = R_FULL // N_CORES  # 1024 rows per core
P = 128
NHI, NLO, NRL, NRH = 32, 128, 4, 32  # c = hi*128+lo ; tile rows = rh*4+rl

F16 = np.float16


def _sylvester(n: int) -> np.ndarray:
    h = np.array([[1.0]], dtype=np.float64)
    while h.shape[0] < n:
        h = np.block([[h, h], [h, -h]])
    return h


def _consts():
    k1 = np.kron(np.eye(NRL), _sylvester(NHI)).astype(F16)
    k2 = (_sylvester(NLO) / 64.0).astype(F16)
    return k1, k2


def build_nc(rows: int = R_CORE):
    assert rows % P == 0
    n_tiles = rows // P

    k1_np, k2_np = _consts()

    nc = bacc.Bacc("TRN2", target_bir_lowering=False, debug=False)
    x_in = nc.dram_tensor("x", [rows, C], mybir.dt.float16, kind="ExternalInput")
    y_out = nc.dram_tensor("y", [rows, C], mybir.dt.float16, kind="ExternalOutput")
    k1_dram = nc.inline_tensor(k1_np, "k1")
    k2_dram = nc.inline_tensor(k2_np, "k2")

    f16 = mybir.dt.float16
    f32 = mybir.dt.float32

    with tile.TileContext(nc) as tc:
        with (
            tc.tile_pool(name="consts", bufs=1) as cpool,
            tc.tile_pool(name="xt", bufs=6) as xt_pool,
            tc.tile_pool(name="w", bufs=3) as w_pool,
            tc.tile_pool(name="o", bufs=3) as o_pool,
            tc.tile_pool(name="ps1", bufs=2, space="PSUM") as ps1_pool,
            tc.tile_pool(name="ps2", bufs=2, space="PSUM") as ps2_pool,
        ):
            k1 = cpool.tile([P, P], f16)
            k2 = cpool.tile([P, P], f16)
            # constants ride the (otherwise idle early) GpSimd SWDGE ring so
            # both HWDGE rings can start streaming x immediately
            nc.gpsimd.dma_start(out=k1[:], in_=k1_dram[:])
            nc.gpsimd.dma_start(out=k2[:], in_=k2_dram[:])

            # Only DVE and ACT can read PSUM; measured drain throughput is
            # ~112 G elem/s on both, so split 4:4.
            eng_p1 = [nc.vector, nc.scalar, nc.vector, nc.scalar]
            eng_p2 = [nc.scalar, nc.vector, nc.scalar, nc.vector]

            def _copy(eng, out, in_):
                if eng is nc.scalar:
                    eng.copy(out=out, in_=in_)
                else:
                    eng.tensor_copy(out=out, in_=in_)

            xts = [None] * n_tiles
            ws = [None] * n_tiles
            ohs = [None] * n_tiles

            def load(t):
                xt = xt_pool.tile([P, C], f16, tag="xt")
                # sub-tile loads: pass1 group g only needs cols
                # [g*1024, (g+1)*1024). Tile 0 loads quarter-wise on the
                # Scalar ring (whose preamble retires ~1us before Sync's) so
                # the first matmul starts as early as possible while Sync
                # streams tile 1 in parallel.
                eng = nc.scalar if t == 0 else nc.sync
                step = 1024 if t == 0 else 2048
                for c0 in range(0, C, step):
                    eng.dma_start(
                        out=xt[:, c0 : c0 + step],
                        in_=x_in[t * P : (t + 1) * P, c0 : c0 + step],
                    )
                xts[t] = xt

            def p1_group(t, g):
                xt = xts[t]
                if g == 0:
                    ws[t] = w_pool.tile([P, C], f16, tag="w", name=f"w{t}")
                w = ws[t]
                ps = ps1_pool.tile([P, 1024], f32)
                for q in range(8):
                    rh = 8 * g + q
                    nc.tensor.matmul(
                        ps[:, q * P : (q + 1) * P],
                        lhsT=xt[:, rh * P : (rh + 1) * P],
                        rhs=k1[:],
                        start=True,
                        stop=True,
                    )
                _copy(eng_p1[g], w[:, g * 1024 : (g + 1) * 1024], ps[:])

            def p2_group(t, g):
                # k2 is the stationary operand here; w streams as the moving
                # operand in N=512 chunks (no per-MM weight reload). Output
                # partition becomes l, free keeps the (rh, rl, j) indexing.
                w = ws[t]
                if g == 0:
                    ohs[t] = o_pool.tile([P, C], f16, tag="oh", name=f"oh{t}")
                oh = ohs[t]
                ps = ps2_pool.tile([P, 1024], f32)
                for h in range(2):
                    n0 = g * 1024 + h * 512
                    nc.tensor.matmul(
                        ps[:, h * 512 : (h + 1) * 512],
                        lhsT=k2[:],
                        rhs=w[:, n0 : n0 + 512],
                        start=True,
                        stop=True,
                    )
                _copy(eng_p2[g], oh[:, g * 1024 : (g + 1) * 1024], ps[:])
                # store each half as soon as its two drains land. The last
                # tile stores quarter-wise on the (by then idle) sync HWDGE
                # ring so the tail is just one quarter-transfer.
                if t == n_tiles - 1:
                    nc.sync.dma_start(
                        out=y_out[t * P : (t + 1) * P, g * 1024 : (g + 1) * 1024],
                        in_=oh[:, g * 1024 : (g + 1) * 1024],
                    )
                elif g % 2 == 1:
                    hh = g // 2
                    nc.gpsimd.dma_start(
                        out=y_out[t * P : (t + 1) * P, hh * 2048 : (hh + 1) * 2048],
                        in_=oh[:, hh * 2048 : (hh + 1) * 2048],
                    )

            # One-tile software skew: pass2 of tile t-1 issues after pass1 of
            # tile t, so pass-2 matmuls never wait on fresh pass-1 drains.
            for t in range(n_tiles):
                load(t)
                for g in range(4):
                    p1_group(t, g)
                if t >= 1:
                    for g in range(4):
                        p2_group(t - 1, g)
            for g in range(4):
                p2_group(n_tiles - 1, g)

    nc.compile()
    nc.finalize()
    return nc


_NC_CACHE: dict[int, object] = {}


def _get_nc(rows: int):
    if rows not in _NC_CACHE:
        _NC_CACHE[rows] = build_nc(rows)
    return _NC_CACHE[rows]


def _prep_inputs(x: np.ndarray, seed: np.ndarray) -> np.ndarray:
    """Fold the seed multiply and transpose to the device SBUF layout.

    Returns [N_CORES, R_CORE, C] fp16 where, per core,
    row = tile*128 + rl*32 + hi and col = rh*128 + lo maps source row
    tile*128 + rh*4 + rl, source col hi*128 + lo.
    """
    xs = (np.asarray(x, dtype=np.float32) * np.asarray(seed, dtype=np.float32)).astype(
        F16
    )
    t = xs.reshape(N_CORES, R_CORE // P, NRH, NRL, NHI, NLO)
    t = np.ascontiguousarray(t.transpose(0, 1, 3, 4, 2, 5))
    return t.reshape(N_CORES, R_CORE, C)


def _post_output(parts: list[np.ndarray]) -> np.ndarray:
    """Invert the device output layout: per core, device row = tile*128 + l
    and col = rh*128 + rl*32 + j holds out[tile*128 + rh*4 + rl, j*128+l]."""
    o = np.stack(parts, axis=0).reshape(N_CORES, R_CORE // P, NLO, NRH, NRL, NHI)
    o = o.transpose(0, 1, 3, 4, 5, 2)
    return np.ascontiguousarray(o).reshape(R_FULL, C).astype(np.float32)


def run(x: np.ndarray, seed: np.ndarray, trace: bool = False):
    nc = _get_nc(R_CORE)
    xp = _prep_inputs(x, seed)
    in_maps = [{"x": xp[i]} for i in range(N_CORES)]
    res = run_bass_kernel_spmd(nc, in_maps, core_ids=list(range(N_CORES)), trace=trace)
    out = _post_output([res.results[i]["y"] for i in range(N_CORES)])
    return out, res


def kernel(x: np.ndarray, seed: np.ndarray) -> np.ndarray:
    out, _ = run(x, seed)
    return out


# revision 23
# speedup vs baseline: 1.0782x; 1.0782x over previous
"""Randomized Hadamard transform kernel for Trainium2 (8 NeuronCores, SPMD).

Math: out = FWHT(x * seed) / sqrt(4096); the reference butterfly equals the
Sylvester Hadamard matrix, and H_4096 = H_32 (x) H_128 (c = hi*128 + lo):

    out[r, j*128 + l] = (1/64) * sum_{hi,lo} H32[hi,j] H128[lo,l] x~[r, hi*128+lo]

Layout trick: matmul computes out[m, n] = sum_k lhsT[k, m] * rhs[k, n].
With the *data* as stationary lhsT and the Hadamard factor as moving rhs,
one MM both contracts the data's partition index and rotates a 128-wide
free window onto the output partitions. Two such passes apply both factors
and land in a store-friendly orientation — no explicit transposes.

This version keeps the baseline's matmul structure but moves the whole
datapath to fp16 (PE: 1 cycle/row vs fp32's 4; DMA bytes halved) and hoists
the seed multiply plus both layout shuffles to the host, so every device DMA
is a fully contiguous [128 x 4096] block (8 KB per partition line):

  host   xp[(t,rl,hi), (rh,lo)] = x[128t+4rh+rl, 128hi+lo] * seed[128hi+lo]
  tile   load xt <- xp  (1 DMA, contiguous)
  pass1  psum[lo, (rl,j)]  = sum_(rl,hi) xt * (I4 (x) H32)   per rh
  pass2  psum[(rl,j), l]   = sum_lo    w  * (H128/64)        per rh
  store  yp[(t,rl,j), (rh,l)] -> out  (1 DMA, contiguous)
  host   out[128t+4rh+rl, 128j+l] = yp

Pass2 of tile t is issued after pass1 of tile t+1 (one-tile software skew)
so the in-order PE queue never waits on the PSUM-drain copies, which are
rotated across the DVE / Activation / Pool engines.
"""

import numpy as np

import concourse.mybir as mybir
from concourse import bacc
import concourse.tile as tile
from concourse.bass_utils import run_bass_kernel_spmd

N_CORES = 8
R_FULL = 8192
C = 4096
R_CORE = R_FULL // N_CORES  # 1024 rows per core
P = 128
NHI, NLO, NRL, NRH = 32, 128, 4, 32  # c = hi*128+lo ; tile rows = rh*4+rl

F16 = np.float16


def _sylvester(n: int) -> np.ndarray:
    h = np.array([[1.0]], dtype=np.float64)
    while h.shape[0] < n:
        h = np.block([[h, h], [h, -h]])
    return h


def _consts():
    k1 = np.kron(np.eye(NRL), _sylvester(NHI)).astype(F16)
    k2 = (_sylvester(NLO) / 64.0).astype(F16)
    return k1, k2


def build_nc(rows: int = R_CORE):
    assert rows % P == 0
    n_tiles = rows // P

    k1_np, k2_np = _consts()

    nc = bacc.Bacc("TRN2", target_bir_lowering=False, debug=False)
    x_in = nc.dram_tensor("x", [rows, C], mybir.dt.float16, kind="ExternalInput")
    y_out = nc.dram_tensor("y", [rows, C], mybir.dt.float16, kind="ExternalOutput")
    k1_dram = nc.inline_tensor(k1_np, "k1")
    k2_dram = nc.inline_tensor(k2_np, "k2")

    f16 = mybir.dt.float16
    f32 = mybir.dt.float32

    with tile.TileContext(nc) as tc:
        with (
            tc.tile_pool(name="consts", bufs=1) as cpool,
            tc.tile_pool(name="xt", bufs=6) as xt_pool,
            tc.tile_pool(name="w", bufs=3) as w_pool,
            tc.tile_pool(name="o", bufs=3) as o_pool,
            tc.tile_pool(name="ps1", bufs=2, space="PSUM") as ps1_pool,
            tc.tile_pool(name="ps2", bufs=2, space="PSUM") as ps2_pool,
        ):
            k1 = cpool.tile([P, P], f16)
            k2 = cpool.tile([P, P], f16)
            # constants ride the Scalar HWDGE ring so the x loads
            # start immediately on the Sync ring
            nc.scalar.dma_start(out=k1[:], in_=k1_dram[:])
            nc.scalar.dma_start(out=k2[:], in_=k2_dram[:])

            # Only DVE and ACT can read PSUM; measured drain throughput is
            # ~112 G elem/s on both, so split 4:4.
            eng_p1 = [nc.vector, nc.scalar, nc.vector, nc.scalar]
            eng_p2 = [nc.scalar, nc.vector, nc.scalar, nc.vector]

            def _copy(eng, out, in_):
                if eng is nc.scalar:
                    eng.copy(out=out, in_=in_)
                else:
                    eng.tensor_copy(out=out, in_=in_)

            xts = [None] * n_tiles
            ws = [None] * n_tiles
            ohs = [None] * n_tiles

            def load(t):
                xt = xt_pool.tile([P, C], f16, tag="xt")
                # sub-tile loads: pass1 group g only needs cols
                # [g*1024, (g+1)*1024). Tile 0 loads in quarters so the first
                # matmul starts as early as possible.
                step = 1024 if t == 0 else 2048
                for c0 in range(0, C, step):
                    nc.sync.dma_start(
                        out=xt[:, c0 : c0 + step],
                        in_=x_in[t * P : (t + 1) * P, c0 : c0 + step],
                    )
                xts[t] = xt

            def p1_group(t, g):
                xt = xts[t]
                if g == 0:
                    ws[t] = w_pool.tile([P, C], f16, tag="w", name=f"w{t}")
                w = ws[t]
                ps = ps1_pool.tile([P, 1024], f32)
                for q in range(8):
                    rh = 8 * g + q
                    nc.tensor.matmul(
                        ps[:, q * P : (q + 1) * P],
                        lhsT=xt[:, rh * P : (rh + 1) * P],
                        rhs=k1[:],
                        start=True,
                        stop=True,
                    )
                _copy(eng_p1[g], w[:, g * 1024 : (g + 1) * 1024], ps[:])

            def p2_group(t, g):
                # k2 is the stationary operand here; w streams as the moving
                # operand in N=512 chunks (no per-MM weight reload). Output
                # partition becomes l, free keeps the (rh, rl, j) indexing.
                w = ws[t]
                if g == 0:
                    ohs[t] = o_pool.tile([P, C], f16, tag="oh", name=f"oh{t}")
                oh = ohs[t]
                ps = ps2_pool.tile([P, 1024], f32)
                for h in range(2):
                    n0 = g * 1024 + h * 512
                    nc.tensor.matmul(
                        ps[:, h * 512 : (h + 1) * 512],
                        lhsT=k2[:],
                        rhs=w[:, n0 : n0 + 512],
                        start=True,
                        stop=True,
                    )
                _copy(eng_p2[g], oh[:, g * 1024 : (g + 1) * 1024], ps[:])
                # store each half as soon as its two drains land. The last
                # tile stores quarter-wise on the (by then idle) sync HWDGE
                # ring so the tail is just one quarter-transfer.
                if t == n_tiles - 1:
                    nc.sync.dma_start(
                        out=y_out[t * P : (t + 1) * P, g * 1024 : (g + 1) * 1024],
                        in_=oh[:, g * 1024 : (g + 1) * 1024],
                    )
                elif g % 2 == 1:
                    hh = g // 2
                    nc.gpsimd.dma_start(
                        out=y_out[t * P : (t + 1) * P, hh * 2048 : (hh + 1) * 2048],
                        in_=oh[:, hh * 2048 : (hh + 1) * 2048],
                    )

            # One-tile software skew: pass2 of tile t-1 issues after pass1 of
            # tile t, so pass-2 matmuls never wait on fresh pass-1 drains.
            for t in range(n_tiles):
                load(t)
                for g in range(4):
                    p1_group(t, g)
                if t >= 1:
                    for g in range(4):
                        p2_group(t - 1, g)
            for g in range(4):
                p2_group(n_tiles - 1, g)

    nc.compile()
    nc.finalize()
    return nc


_NC_CACHE: dict[int, object] = {}


def _get_nc(rows: int):
    if rows not in _NC_CACHE:
        _NC_CACHE[rows] = build_nc(rows)
    return _NC_CACHE[rows]


def _prep_inputs(x: np.ndarray, seed: np.ndarray) -> np.ndarray:
    """Fold the seed multiply and transpose to the device SBUF layout.

    Returns [N_CORES, R_CORE, C] fp16 where, per core,
    row = tile*128 + rl*32 + hi and col = rh*128 + lo maps source row
    tile*128 + rh*4 + rl, source col hi*128 + lo.
    """
    xs = (np.asarray(x, dtype=np.float32) * np.asarray(seed, dtype=np.float32)).astype(
        F16
    )
    t = xs.reshape(N_CORES, R_CORE // P, NRH, NRL, NHI, NLO)
    t = np.ascontiguousarray(t.transpose(0, 1, 3, 4, 2, 5))
    return t.reshape(N_CORES, R_CORE, C)


def _post_output(parts: list[np.ndarray]) -> np.ndarray:
    """Invert the device output layout: per core, device row = tile*128 + l
    and col = rh*128 + rl*32 + j holds out[tile*128 + rh*4 + rl, j*128+l]."""
    o = np.stack(parts, axis=0).reshape(N_CORES, R_CORE // P, NLO, NRH, NRL, NHI)
    o = o.transpose(0, 1, 3, 4, 5, 2)
    return np.ascontiguousarray(o).reshape(R_FULL, C).astype(np.float32)


def run(x: np.ndarray, seed: np.ndarray, trace: bool = False):
    nc = _get_nc(R_CORE)
    xp = _prep_inputs(x, seed)
    in_maps = [{"x": xp[i]} for i in range(N_CORES)]
    res = run_bass_kernel_spmd(nc, in_maps, core_ids=list(range(N_CORES)), trace=trace)
    out = _post_output([res.results[i]["y"] for i in range(N_CORES)])
    return out, res


def kernel(x: np.ndarray, seed: np.ndarray) -> np.ndarray:
    out, _ = run(x, seed)
    return out
